# revision 17
# baseline (speedup 1.0000x reference)
"""ALIF neuron rollout (T=100, B=64, J=512, K=1024) on 8 TRN2 NeuronCores.

Strategy (per core, data-parallel over batch, 8 batches/core):
  1. The synaptic-current recurrence (syn_exc/syn_teach share one epsp decay,
     syn_inh is identically zero) is LINEAR, so it folds into TensorE as a
     [T,T] lower-triangular Toeplitz filter applied to the INPUTS:
       xf = L1s-filter(x)   (32 small matmuls, lhsT = x in natural layout)
       S[t,k] = xf^T @ (w_eff*j_eff)  +  L2s @ teacher
     The elementwise weight product runs on VectorE over streamed chunks.
  2. S (the scaled drive) is PE-transposed to [k,t] layout, k = kt + 8*p.
  3. The nonlinear threshold scan runs in the u_t = ab^-t rescaled domain,
     which makes the adaptation state a PURE ACCUMULATOR (no per-step decay
     op) at the cost of a constant kappa = dmc/ab factor on the membrane
     carry, absorbed by a custom DVE op. Two custom VectorE ops + two Pool
     (gpsimd) ops per step:
        W_t     = select(P_t > Q_t + G_t, -BIG, P_t)      [V, custom]
        P_{t+1} = relu(W_t)*kappa + drive_{t+1}           [V, custom]
        spk_t   = (P_t - G_t) > Q_t                        [Pool, stt]
        Q_{t+1} = spk_t*c_t + Q_t                          [Pool, stt]
     G_t = thr0*ab^-t, c_t = beta*ab^-t are per-step immediates; spikes DMA
     out every 2 steps from the Pool-written staging tile.
  Layout: k = kt + 8*p (p = partition 0..127, kt = 0..7), state free dim =
  (b*8 + kt) so spikes DMA out k-contiguously per (t, b).
"""
import numpy as np

import concourse.bass as bass
import concourse.tile as tile
from concourse import bacc, mybir
from concourse.bass_utils import run_bass_kernel_spmd

T, B, J, K = 100, 64, 512, 1024
DT = 1.0
NCORES = 8
BLOC = B // NCORES           # 8 batches per core
NKT = 8                      # k interleave factor: k = kt + 8*p
NSLOT = BLOC * NKT           # 64 state slots (free dim of scan tiles)
BIG = 1.0e30
F32 = mybir.dt.float32


# ---------------------------------------------------------------------------
# Custom DVE ops (registered into concourse.dve_ops at import time; the
# per-NEFF table is generated from these specs at compile time).
def _dve_relu_np(x):
    return np.maximum(np.nan_to_num(x, nan=0.0, posinf=np.inf,
                                    neginf=-np.inf), 0)


def _register_dve(name, spec):
    import concourse.dve_ops as dops
    from concourse.dve_spec import lower, _has_src1
    from concourse.dve_uop import DveOpSpec
    if name in dops._SUB_OPCODE_FOR_NAME:
        return next(op for op in dops.OPS if op.name == name)
    row = dops._CUSTOM_DVE_ROW_BASE + len(dops.OPS)
    assert row < 0x20, "custom DVE row budget exhausted"
    shas = {}
    for ver in ("v3", "v4"):
        s = DveOpSpec(name=name, opcode=row, uops=lower(spec, ver=ver),
                      rd1_en=_has_src1(spec))
        shas[ver] = s.sha(ver)
    op = dops.DveOp(name, spec, subdim=False, uops_sha=shas)
    dops.OPS.append(op)
    dops.CUSTOM_DVE_SPECS[name] = spec
    dops._SUB_OPCODE_FOR_NAME[name] = row
    return op


def _alif_ops():
    from concourse.dve_spec import Spec, Src0, Src1, C0, C1, Zero, relu, select
    carry = _register_dve(
        "ALIF_CARRY",
        Spec(
            body=select(Src0 > Src1, Zero, relu(Src0 + C1) * C0),
            reference=lambda in0, in1, s0, s1, imm2: np.where(
                in0 > in1, np.float32(0.0),
                _dve_relu_np(in0 + s1) * s0).astype(np.float32),
        ),
    )
    cspk = _register_dve(
        "ALIF_CSPK",
        Spec(
            body=select(Src0 > Src1, C0, Zero),
            reference=lambda in0, in1, s0, s1, imm2: np.where(
                in0 > in1, np.float32(s0), np.float32(0.0)
            ).astype(np.float32),
        ),
    )
    return carry, cspk


def _scalar(v, name):
    v = np.asarray(v, np.float64)
    if v.ndim == 0:
        return float(v)
    if np.ptp(v) != 0.0:
        raise NotImplementedError(f"{name} must be uniform for this kernel")
    return float(v.reshape(-1)[0])


def _host_constants(w_teach, tau_mem, tau_adapt, tau_epsp, thr_0, beta_adapt):
    dm = DT / _scalar(tau_mem, "tau_mem")
    dmc = 1.0 - dm
    da = DT / _scalar(tau_adapt, "tau_adapt")
    ab = 1.0 - da
    thr0 = _scalar(thr_0, "thr_0")
    assert thr0 > 0.0, "kernel assumes thr_0 > 0 (spike compare without relu)"
    beta = _scalar(beta_adapt, "beta_adapt")
    epsp = 1.0 - DT / _scalar(tau_epsp, "tau_epsp")
    wt = _scalar(w_teach, "w_teach")

    u = ab ** (-np.arange(T + 1, dtype=np.float64))      # u_t = ab^-t
    kappa = dmc / ab
    g_bias = thr0 * u[:T]                                # G_t
    c_acc = beta * u[:T]                                 # c_t

    tt_, tau_ = np.meshgrid(np.arange(T), np.arange(T), indexing="ij")
    base = np.where(tau_ <= tt_ - 1,
                    epsp ** np.maximum(tt_ - 1 - tau_, 0), 0.0)
    l1 = (u[:T, None] * dm * base).astype(np.float32)    # [t, tau]
    l2 = (u[:T, None] * dm * wt * base).astype(np.float32)
    l1t = np.ascontiguousarray(l1.T)                     # [tau, t]
    l2t = np.ascontiguousarray(l2.T)

    gcol = (-g_bias).astype(np.float32).reshape(T, 1)    # drive bias: -G_t
    ident = np.eye(T, dtype=np.float32)
    return dict(kappa=kappa, g_bias=g_bias, c_acc=c_acc,
                l1t=l1t, l2t=l2t, ident=ident, gcol=gcol)


def build_program(consts):
    """One SPMD program; all 8 cores run it on their own batch shard."""
    kappa = float(consts["kappa"])
    g_bias = consts["g_bias"]
    c_acc = consts["c_acc"]
    assert np.all(np.asarray(c_acc) > 0.0), \
        "spike extraction via Sign(c*spk) needs beta_adapt > 0"
    CARRY, CSPK = _alif_ops()
    nc = bacc.Bacc("TRN2", target_bir_lowering=False, debug=False,
                   num_devices=NCORES)

    x_h = nc.declare_dram_parameter("x", [T, BLOC, J], F32, isOutput=False)
    te_h = nc.declare_dram_parameter("teacher", [T, BLOC, K], F32,
                                     isOutput=False)
    we_h = nc.declare_dram_parameter("w_eff", [BLOC, J, K], F32,
                                     isOutput=False)
    je_h = nc.declare_dram_parameter("j_eff", [BLOC, J, K], F32,
                                     isOutput=False)
    l1_h = nc.declare_dram_parameter("l1t", [T, T], F32, isOutput=False)
    l2_h = nc.declare_dram_parameter("l2t", [T, T], F32, isOutput=False)
    id_h = nc.declare_dram_parameter("ident", [T, T], F32, isOutput=False)
    gc_h = nc.declare_dram_parameter("gcol", [T, 1], F32, isOutput=False)
    out_h = nc.declare_dram_parameter("out", [T, BLOC, K], F32, isOutput=True)

    JT = J // 128            # 4 j-tiles
    WCHUNKS = [1, 1, 2, 2, 2]    # w_eff chunk sizes (batches)

    from contextlib import ExitStack
    with tile.TileContext(nc) as tc, ExitStack() as ctx:
        cpool = ctx.enter_context(tc.tile_pool(name="consts", bufs=1))
        xpool = ctx.enter_context(tc.tile_pool(name="x", bufs=1))
        xtpool = ctx.enter_context(tc.tile_pool(name="xt", bufs=1))
        wpool = ctx.enter_context(tc.tile_pool(name="w", bufs=2))
        jpool = ctx.enter_context(tc.tile_pool(name="j", bufs=3))
        tpool = ctx.enter_context(tc.tile_pool(name="teach", bufs=2))
        hpool = ctx.enter_context(tc.tile_pool(name="h", bufs=2))
        dpool = ctx.enter_context(tc.tile_pool(name="drive", bufs=1))
        spool = ctx.enter_context(tc.tile_pool(name="spk", bufs=8))
        scpool = ctx.enter_context(tc.tile_pool(name="scan", bufs=3))
        ps_h = ctx.enter_context(tc.tile_pool(name="psH", bufs=2,
                                              space="PSUM"))
        ps_d = ctx.enter_context(tc.tile_pool(name="psD", bufs=2,
                                              space="PSUM"))
        ps_x = ctx.enter_context(tc.tile_pool(name="psX", bufs=2,
                                              space="PSUM"))

        l1t_sb = cpool.tile([T, T], F32, tag="l1")
        l2t_sb = cpool.tile([T, T], F32, tag="l2")
        id_sb = cpool.tile([T, T], F32, tag="id")
        gc_sb = cpool.tile([T, 1], F32, tag="gc")
        nc.sync.dma_start(l1t_sb[:], l1_h.ap()[:])

        # --- x-filter fold: xf[j,t] = sum_tau x[tau,j] * L1s[t,tau]
        x_sb = xpool.tile([T, BLOC, J], F32, tag="x")
        nc.sync.dma_start(x_sb[:], x_h.ap()[:])
        nc.sync.dma_start(l2t_sb[:], l2_h.ap()[:])
        nc.sync.dma_start(id_sb[:], id_h.ap()[:])
        nc.sync.dma_start(gc_sb[:], gc_h.ap()[:])
        xt_sb = xtpool.tile([128, BLOC * JT, T], F32, tag="xt")
        for b in range(BLOC):
            for jt in range(JT):
                xp = ps_x.tile([128, T], F32, tag="xps")
                nc.tensor.matmul(xp[:],
                                 lhsT=x_sb[:, b, jt * 128:(jt + 1) * 128],
                                 rhs=l1t_sb[:], start=True, stop=True)
                nc.scalar.copy(xt_sb[:, b * JT + jt, :], xp[:])

        # --- drive tiles: [128 (p), 64 (b*8+kt), 100 (t)]
        drive_sb = dpool.tile([128, NSLOT, T], F32, tag="drive")

        b0 = 0
        for cb_ in WCHUNKS:
            bsl = slice(b0, b0 + cb_)
            we_t = wpool.tile([128, 2, JT, K], F32, tag="weff")
            te_t = tpool.tile([T, 2, K], F32, tag="teach")
            we_src = we_h.ap()[bsl].rearrange("b (jt p) k -> p b jt k", p=128)
            if cb_ == 1:
                # split by jt-half so the tail work can start before the
                # final bytes land
                nc.sync.dma_start(we_t[:, :1, :2], we_src[:, :, :2])
                nc.sync.dma_start(we_t[:, :1, 2:], we_src[:, :, 2:])
            else:
                nc.sync.dma_start(we_t[:, :cb_], we_src)
            nc.sync.dma_start(te_t[:, :cb_], te_h.ap()[:, bsl, :])

            for i in range(cb_):
                b = b0 + i
                hps = ps_h.tile([T, K], F32, tag="hps")
                for jh in range(2):                 # jt-halves: jt in {0,1},{2,3}
                    jts = slice(jh * 2, jh * 2 + 2)
                    je_t = jpool.tile([128, 2, K], F32, tag="jeff")
                    nc.sync.dma_start(
                        je_t[:], je_h.ap()[b].rearrange(
                            "(jt p) k -> p jt k", p=128)[:, jts])

                    # w = w_eff * j_eff, in place
                    wf = we_t[:, i, jts].rearrange("p jt k -> p (jt k)")
                    jf = je_t[:].rearrange("p jt k -> p (jt k)")
                    nc.vector.tensor_tensor(wf, wf, jf, mybir.AluOpType.mult)

                    for half in range(2):
                        ksl = slice(half * 512, (half + 1) * 512)
                        for jt in range(jh * 2, jh * 2 + 2):
                            nc.tensor.matmul(
                                hps[:, ksl],
                                lhsT=xt_sb[:, b * JT + jt, :],
                                rhs=we_t[:, i, jt, ksl],
                                start=(jt == 0), stop=False)
                # S[t,k] += teacher part
                for half in range(2):
                    ksl = slice(half * 512, (half + 1) * 512)
                    nc.tensor.matmul(
                        hps[:, ksl],
                        lhsT=l2t_sb[:],
                        rhs=te_t[:, i, ksl],
                        start=False, stop=True)
                hsb = hpool.tile([T, K], F32, tag="hsb")
                # drive in compare-ready form: d_hat[t] = d[t] - G_t
                nc.scalar.activation(hsb[:], hps[:],
                                     mybir.ActivationFunctionType.Identity,
                                     bias=gc_sb[:, 0:1], scale=1.0)

                # transpose drive [t,k] -> [k,t] per kt slice (k = kt + 8*p)
                skt = hsb[:].rearrange("t (p kt) -> t kt p", kt=NKT)
                for kt in range(NKT):
                    dps = ps_d.tile([128, T], F32, tag="dps")
                    nc.tensor.transpose(dps[:], skt[:, kt, :], id_sb[:])
                    nc.scalar.copy(drive_sb[:, b * NKT + kt, :], dps[:])
            b0 += cb_

        # --- the sequential threshold scan (u-domain, P-hat = P - G baked) ---
        # V: CARRY (masked relu*kappa) + CSPK (c_t * spike); Pool: two adds
        # (P' = C + drive, Q' = cspk + Q); Scalar: Sign(cspk) -> 0/1 staging.
        q_prev = scpool.tile([128, NSLOT], F32, tag="Q")
        nc.vector.memset(q_prev[:], 0.0)
        out_r = out_h.ap().rearrange("t b (p kt) -> t p b kt", kt=NKT)
        p_prev = None           # P_0 aliases drive_sb[:, :, 0]
        spk2 = None
        for t in range(T):
            p_in = drive_sb[:, :, 0] if t == 0 else p_prev[:]
            if t % 2 == 0:
                spk2 = spool.tile([128, 2, NSLOT], F32, tag="spk")
            spk = spk2[:, t % 2, :]
            cs_t = scpool.tile([128, NSLOT], F32, tag="CS")
            nc.vector._custom_dve(
                CSPK, out=cs_t[:], in0=p_in, in1=q_prev[:],
                s0=float(c_acc[t]))
            nc.scalar.activation(spk[:], cs_t[:],
                                 mybir.ActivationFunctionType.Sign)
            if t < T - 1:
                c_t = scpool.tile([128, NSLOT], F32, tag="C")
                nc.vector._custom_dve(
                    CARRY, out=c_t[:], in0=p_in, in1=q_prev[:],
                    s0=kappa, s1=float(g_bias[t]))
                p_new = scpool.tile([128, NSLOT], F32, tag="P")
                nc.gpsimd.tensor_tensor(p_new[:], c_t[:],
                                        drive_sb[:, :, t + 1],
                                        mybir.AluOpType.add)
                q_new = scpool.tile([128, NSLOT], F32, tag="Q")
                nc.gpsimd.tensor_tensor(q_new[:], cs_t[:], q_prev[:],
                                        mybir.AluOpType.add)
                p_prev, q_prev = p_new, q_new
            if t % 2 == 1:
                nc.sync.dma_start(
                    out_r[t - 1:t + 1].rearrange("t p b kt -> p t b kt"),
                    spk2[:].rearrange("p t (b kt) -> p t b kt", kt=NKT))

    nc.compile()
    return nc


def _prepare(inputs):
    x = np.ascontiguousarray(np.asarray(inputs["x"], np.float32))
    teacher = np.ascontiguousarray(np.asarray(inputs["teacher"], np.float32))
    w_eff = np.ascontiguousarray(np.asarray(inputs["w_eff"], np.float32))
    j_eff = np.ascontiguousarray(np.asarray(inputs["j_eff"], np.float32))
    consts = _host_constants(
        inputs["w_teach"], inputs["tau_mem"], inputs["tau_adapt"],
        inputs["tau_epsp"], inputs["thr_0"], inputs["beta_adapt"])
    in_maps = []
    for i in range(NCORES):
        sl = slice(i * BLOC, (i + 1) * BLOC)
        in_maps.append({
            "x": np.ascontiguousarray(x[:, sl]),
            "teacher": np.ascontiguousarray(teacher[:, sl]),
            "w_eff": np.ascontiguousarray(w_eff[sl]),
            "j_eff": np.ascontiguousarray(j_eff[sl]),
            "l1t": consts["l1t"], "l2t": consts["l2t"],
            "ident": consts["ident"], "gcol": consts["gcol"],
        })
    return consts, in_maps


def run(inputs, trace=False, **kw):
    consts, in_maps = _prepare(inputs)
    nc = build_program(consts)
    res = run_bass_kernel_spmd(nc, in_maps, core_ids=list(range(NCORES)),
                               trace=trace, **kw)
    out = np.concatenate([res.results[i]["out"] for i in range(NCORES)],
                         axis=1)
    return out.astype(np.float32), res


def kernel(**inputs) -> np.ndarray:
    out, _ = run(inputs)
    return out
